# revision 8
# baseline (speedup 1.0000x reference)
"""TopK sparse autoencoder on 8 Trainium2 NeuronCores.

Data-parallel over batch: each core handles 1024 of 8192 rows.
Per core:
  Phase A : pre = xc @ enc + lb via fp16 triple-split matmuls
            (xh@eh + 2^-11*(xl@eh) + 2^-12*(xh@el), fp32-grade accuracy),
            relu -> fl written fp32 (output) + bf16 (decode scratch).
  Phase A2: per 128-row block: top-32 via 4x(max8 + match_replace),
            lat = fl - zapped; lat written fp32 (output) + bf16 (scratch).
  Phase B : rec_full = fl @ dec + pb, rec = lat @ dec + pb as bf16 matmuls
            contracting over L with xbar-transposed bf16 tiles.
"""
import os
import sys

for _p in ("/opt/trn_rl_repo", "/root/.axon_site/_ro/trn_rl_repo"):
    if os.path.isdir(_p) and _p not in sys.path:
        sys.path.append(_p)

import numpy as np
import ml_dtypes

import concourse.tile as tile
from concourse import bacc, mybir
from concourse.bass_utils import run_bass_kernel_spmd

P = 128
B = 8192          # full batch
D = 2048          # model dim
L = 16384         # latents
NCORES = 8
BC = B // NCORES  # rows per core = 1024
MT = BC // P      # m-tiles per core = 8
KC = D // P       # k-chunks for encode = 16
NT = L // 512     # n-tiles for encode = 32
LKB = 1024        # L-block for decode
NKB = L // LKB    # 16
LKC = LKB // P    # k-chunks per decode block = 8

XL_SCALE = 2.0 ** 11   # xl shipped as (xc - xh) * 2^11
EL_SCALE = 2.0 ** 12   # el shipped as (enc - eh) * 2^12

f32 = mybir.dt.float32
bf16 = mybir.dt.bfloat16
f16 = mybir.dt.float16

_COMPILED = None


def _build():
    nc = bacc.Bacc("TRN2", target_bir_lowering=False, debug=False,
                   num_devices=NCORES)

    xh = nc.dram_tensor("xh", [D, BC], f16, kind="ExternalInput").ap()
    xl = nc.dram_tensor("xl", [D, BC], f16, kind="ExternalInput").ap()
    eh = nc.dram_tensor("eh", [D, L], f16, kind="ExternalInput").ap()
    el = nc.dram_tensor("el", [D, L], f16, kind="ExternalInput").ap()
    lbh = nc.dram_tensor("lbh", [1, L], bf16, kind="ExternalInput").ap()
    lbl = nc.dram_tensor("lbl", [1, L], bf16, kind="ExternalInput").ap()
    dec_bf = nc.dram_tensor("dec_bf", [L, D], bf16, kind="ExternalInput").ap()
    pb_rep = nc.dram_tensor("pb_rep", [P, D], f32, kind="ExternalInput").ap()

    fl_o = nc.dram_tensor("fl", [BC, L], f32, kind="ExternalOutput").ap()
    lat_o = nc.dram_tensor("lat", [BC, L], f32, kind="ExternalOutput").ap()
    rec_o = nc.dram_tensor("rec", [BC, D], f32, kind="ExternalOutput").ap()
    recf_o = nc.dram_tensor("recf", [BC, D], f32, kind="ExternalOutput").ap()

    flbf_d = nc.dram_tensor("flbf_d", [BC, L], bf16).ap()
    latbf_d = nc.dram_tensor("latbf_d", [BC, L], bf16).ap()

    with tile.TileContext(nc) as tc:
        # ======== Phase A + A2 interleaved over m-tile pairs ========
        # Encoder is re-streamed once per pair (4x total); each pair's
        # DVE top-k work hides under the next pair's PE encode.
        with (
            tc.tile_pool(name="resA", bufs=1) as resA,
            tc.tile_pool(name="xp", bufs=2) as xp,
            tc.tile_pool(name="ehp", bufs=2) as ehp,
            tc.tile_pool(name="elp", bufs=1) as elp,
            tc.tile_pool(name="evA", bufs=2) as evA,
            tc.tile_pool(name="flp", bufs=1) as flp,
            tc.tile_pool(name="ck", bufs=2) as ck,
            tc.tile_pool(name="psA", bufs=2, space="PSUM") as psA,
        ):
            t_ones = resA.tile([1, P], bf16)
            nc.vector.memset(t_ones[:], 1.0)

            for mp in range(MT // 2):
                t_xh = xp.tile([P, KC, 2, P], f16, tag="xh")
                nc.sync.dma_start(
                    t_xh[:], xh[:, mp * 256:(mp + 1) * 256].rearrange(
                        "(kc p) (mt mm) -> p kc mt mm", p=P, mm=P))
                t_xl = xp.tile([P, KC, 2, P], f16, tag="xl")
                nc.sync.dma_start(
                    t_xl[:], xl[:, mp * 256:(mp + 1) * 256].rearrange(
                        "(kc p) (mt mm) -> p kc mt mm", p=P, mm=P))
                for nt in range(NT):
                    nsl = slice(nt * 512, (nt + 1) * 512)
                    eh_blk = ehp.tile([P, KC, 512], f16, tag="ehblk")
                    nc.sync.dma_start(
                        eh_blk[:], eh[:, nsl].rearrange("(kc p) n -> p kc n", p=P))
                    el_blk = elp.tile([P, KC, 512], f16, tag="elblk")
                    nc.sync.dma_start(
                        el_blk[:], el[:, nsl].rearrange("(kc p) n -> p kc n", p=P))
                    t_lbh = ehp.tile([1, 512], bf16, tag="lbh")
                    nc.sync.dma_start(t_lbh[:], lbh[:, nsl])
                    t_lbl = ehp.tile([1, 512], bf16, tag="lbl")
                    nc.sync.dma_start(t_lbl[:], lbl[:, nsl])
                    for mi in range(2):
                        m = mp * 2 + mi
                        ps1 = psA.tile([P, 512], f32, tag="ps1")
                        ps2 = psA.tile([P, 512], f32, tag="ps2")
                        ps3 = psA.tile([P, 512], f32, tag="ps3")
                        for k in range(KC):
                            nc.tensor.matmul(ps1[:], t_xh[:, k, mi, :],
                                             eh_blk[:, k, :],
                                             start=(k == 0), stop=False)
                            nc.tensor.matmul(ps2[:], t_xl[:, k, mi, :],
                                             eh_blk[:, k, :],
                                             start=(k == 0), stop=(k == KC - 1))
                            nc.tensor.matmul(ps3[:], t_xh[:, k, mi, :],
                                             el_blk[:, k, :],
                                             start=(k == 0), stop=(k == KC - 1))
                        nc.tensor.matmul(ps1[:], t_ones[:, :], t_lbh[:, :],
                                         start=False, stop=False)
                        nc.tensor.matmul(ps1[:], t_ones[:, :], t_lbl[:, :],
                                         start=False, stop=True)
                        t1 = evA.tile([P, 512], f32, tag="t1")
                        nc.vector.tensor_scalar_mul(t1[:], ps2[:], 1.0 / XL_SCALE)
                        t2 = evA.tile([P, 512], f32, tag="t2")
                        nc.vector.scalar_tensor_tensor(
                            t2[:], ps3[:], 1.0 / EL_SCALE, t1[:],
                            op0=mybir.AluOpType.mult, op1=mybir.AluOpType.add)
                        t3 = evA.tile([P, 512], f32, tag="t3")
                        nc.vector.tensor_add(t3[:], ps1[:], t2[:])
                        o = evA.tile([P, 512], f32, tag="evict")
                        nc.scalar.activation(o[:], t3[:],
                                             mybir.ActivationFunctionType.Relu)
                        nc.sync.dma_start(fl_o[m * P:(m + 1) * P, nsl], o[:])
                        ob = evA.tile([P, 512], bf16, tag="evictbf")
                        nc.scalar.activation(ob[:], o[:],
                                             mybir.ActivationFunctionType.Copy)
                        nc.sync.dma_start(flbf_d[m * P:(m + 1) * P, nsl], ob[:])

                # top-k for this pair (DVE) — overlaps next pair's encode (PE)
                for mi in range(2):
                    m = mp * 2 + mi
                    msl = slice(m * P, (m + 1) * P)
                    fl_blk = flp.tile([P, L], f32, tag="flblk")
                    nc.sync.dma_start(fl_blk[:], fl_o[msl, :])
                    maxes = ck.tile([P, 32], f32, tag="maxes")
                    for r in range(4):
                        nc.vector.max(out=maxes[:, r * 8:(r + 1) * 8],
                                      in_=fl_blk[:])
                        nc.vector.match_replace(
                            out=fl_blk[:],
                            in_to_replace=maxes[:, r * 8:(r + 1) * 8],
                            in_values=fl_blk[:], imm_value=0.0)
                    for c in range(L // 2048):
                        csl = slice(c * 2048, (c + 1) * 2048)
                        flc = ck.tile([P, 2048], f32, tag="flc")
                        nc.sync.dma_start(flc[:], fl_o[msl, csl])
                        latc = ck.tile([P, 2048], f32, tag="latc")
                        nc.vector.tensor_sub(latc[:], flc[:], fl_blk[:, csl])
                        nc.sync.dma_start(lat_o[msl, csl], latc[:])
                        lbfc = ck.tile([P, 2048], bf16, tag="lbfc")
                        nc.scalar.activation(lbfc[:], latc[:],
                                             mybir.ActivationFunctionType.Copy)
                        nc.sync.dma_start(latbf_d[msl, csl], lbfc[:])

        # ======== Phase B: decode ========
        with (
            tc.tile_pool(name="resB", bufs=1) as resB,
            tc.tile_pool(name="acc", bufs=1) as accp,
            tc.tile_pool(name="decp", bufs=2) as decp,
            tc.tile_pool(name="tp", bufs=2) as tp,
            tc.tile_pool(name="outp", bufs=2) as outp,
            tc.tile_pool(name="psB", bufs=1, space="PSUM") as psB,
        ):
            t_pb = resB.tile([P, D], f32)
            nc.sync.dma_start(t_pb[:], pb_rep)

            for bh in range(2):
                rows = slice(bh * 512, (bh + 1) * 512)
                acc_r = accp.tile([P, 4, D], f32, tag="accr")
                acc_f = accp.tile([P, 4, D], f32, tag="accf")
                for kb in range(NKB):
                    d_blk = decp.tile([P, LKC, D], bf16, tag="decblk")
                    nc.sync.dma_start(
                        d_blk[:],
                        dec_bf[kb * LKB:(kb + 1) * LKB, :].rearrange(
                            "(kc p) d -> p kc d", p=P))
                    fT = tp.tile([P, LKC, 512], bf16, tag="fT")
                    lT = tp.tile([P, LKC, 512], bf16, tag="lT")
                    for j in range(LKC):
                        cl = slice(kb * LKB + j * P, kb * LKB + (j + 1) * P)
                        nc.sync.dma_start(fT[:, j, :], flbf_d[rows, cl],
                                          transpose=True)
                        nc.sync.dma_start(lT[:, j, :], latbf_d[rows, cl],
                                          transpose=True)
                    for mt in range(4):
                        mm = slice(mt * P, (mt + 1) * P)
                        for (tag, src, acc) in (("psf", fT, acc_f),
                                                ("psr", lT, acc_r)):
                            for h in range(2):
                                hsl = slice(h * 1024, (h + 1) * 1024)
                                ps = psB.tile([P, 1024], f32, tag=f"{tag}{h}")
                                for k in range(LKC):
                                    for n in range(2):
                                        nsl = slice(n * 512, (n + 1) * 512)
                                        gsl = slice(h * 1024 + n * 512,
                                                    h * 1024 + (n + 1) * 512)
                                        nc.tensor.matmul(
                                            ps[:, nsl], src[:, k, mm],
                                            d_blk[:, k, gsl],
                                            start=(k == 0), stop=(k == LKC - 1))
                                if kb == 0:
                                    nc.vector.tensor_copy(acc[:, mt, hsl], ps[:])
                                else:
                                    nc.vector.tensor_add(acc[:, mt, hsl],
                                                         acc[:, mt, hsl], ps[:])
                for mt in range(4):
                    grow = slice(bh * 512 + mt * P, bh * 512 + (mt + 1) * P)
                    o_f = outp.tile([P, D], f32, tag="of")
                    nc.vector.tensor_add(o_f[:], acc_f[:, mt, :], t_pb[:])
                    nc.sync.dma_start(recf_o[grow, :], o_f[:])
                    o_r = outp.tile([P, D], f32, tag="orr")
                    nc.vector.tensor_add(o_r[:], acc_r[:, mt, :], t_pb[:])
                    nc.sync.dma_start(rec_o[grow, :], o_r[:])

    nc.compile()
    return nc


def _split_f16(a, scale):
    hi = a.astype(np.float16)
    lo = ((a - hi.astype(np.float32)) * scale).astype(np.float16)
    return hi, lo


def kernel(x, pre_bias, encoder, latent_bias, decoder, k):
    global _COMPILED
    assert int(k) == 32
    x = np.asarray(x, np.float32)
    pre_bias = np.asarray(pre_bias, np.float32)
    encoder = np.ascontiguousarray(np.asarray(encoder, np.float32))
    latent_bias = np.asarray(latent_bias, np.float32)
    decoder = np.asarray(decoder, np.float32)

    xc = x - pre_bias                      # [B, D] fp32, same op as reference
    xcT = np.ascontiguousarray(xc.T)       # [D, B]
    xh_a, xl_a = _split_f16(xcT, XL_SCALE)
    eh_a, el_a = _split_f16(encoder, EL_SCALE)
    lbh_a = latent_bias.astype(ml_dtypes.bfloat16).reshape(1, L)
    lbl_a = (latent_bias - lbh_a.astype(np.float32)).astype(
        ml_dtypes.bfloat16).reshape(1, L)
    dec_bf = decoder.astype(ml_dtypes.bfloat16)
    pb_rep = np.ascontiguousarray(
        np.broadcast_to(pre_bias.reshape(1, D), (P, D)))

    if _COMPILED is None:
        _COMPILED = _build()
    nc = _COMPILED

    in_maps = []
    for c in range(NCORES):
        csl = slice(c * BC, (c + 1) * BC)
        in_maps.append(dict(
            xh=np.ascontiguousarray(xh_a[:, csl]),
            xl=np.ascontiguousarray(xl_a[:, csl]),
            eh=eh_a, el=el_a, lbh=lbh_a, lbl=lbl_a,
            dec_bf=dec_bf, pb_rep=pb_rep))
    kernel.last_in_maps = in_maps

    res = run_bass_kernel_spmd(nc, in_maps, list(range(NCORES)))
    outs = res.results

    rec = np.concatenate([outs[c]["rec"] for c in range(NCORES)], axis=0)
    lat = np.concatenate([outs[c]["lat"] for c in range(NCORES)], axis=0)
    recf = np.concatenate([outs[c]["recf"] for c in range(NCORES)], axis=0)
    fl = np.concatenate([outs[c]["fl"] for c in range(NCORES)], axis=0)
    return rec, lat, recf, fl


if __name__ == "__main__":
    rng = np.random.default_rng(0)
    x = rng.standard_normal((B, D), dtype=np.float32)
    pb = (rng.standard_normal(D) * 0.01).astype(np.float32)
    dec = rng.standard_normal((L, D), dtype=np.float32)
    dec = (dec * (0.1 / np.linalg.norm(dec, axis=-1, keepdims=True))).astype(np.float32)
    en = (dec.T + rng.standard_normal((D, L)) * 0.001).astype(np.float32)
    lb_ = (rng.standard_normal(L) * 0.01).astype(np.float32)
    out = kernel(x=x, pre_bias=pb, encoder=en, latent_bias=lb_, decoder=dec, k=32)
    for o in out:
        print(o.shape, o.dtype)


# revision 10
# speedup vs baseline: 1.0537x; 1.0537x over previous
"""TopK sparse autoencoder on 8 Trainium2 NeuronCores.

Data-parallel over batch: each core handles 1024 of 8192 rows.
Per core:
  Phase A : pre = xc @ enc + lb via fp16 triple-split matmuls
            (xh@eh + 2^-11*(xl@eh) + 2^-12*(xh@el), fp32-grade accuracy),
            relu -> fl written fp32 (output) + bf16 (decode scratch).
  Phase A2: per 128-row block: top-32 via 4x(max8 + match_replace),
            lat = fl - zapped; lat written fp32 (output) + bf16 (scratch).
  Phase B : rec_full = fl @ dec + pb, rec = lat @ dec + pb as bf16 matmuls
            contracting over L with xbar-transposed bf16 tiles.
"""
import os
import sys

for _p in ("/opt/trn_rl_repo", "/root/.axon_site/_ro/trn_rl_repo"):
    if os.path.isdir(_p) and _p not in sys.path:
        sys.path.append(_p)

import numpy as np
import ml_dtypes

import concourse.tile as tile
from concourse import bacc, mybir
from concourse.bass_utils import run_bass_kernel_spmd

P = 128
B = 8192          # full batch
D = 2048          # model dim
L = 16384         # latents
NCORES = 8
BC = B // NCORES  # rows per core = 1024
MT = BC // P      # m-tiles per core = 8
KC = D // P       # k-chunks for encode = 16
NT = L // 512     # n-tiles for encode = 32
LKB = 1024        # L-block for decode
NKB = L // LKB    # 16
LKC = LKB // P    # k-chunks per decode block = 8

XL_SCALE = 2.0 ** 11   # xl shipped as (xc - xh) * 2^11
EL_SCALE = 2.0 ** 12   # el shipped as (enc - eh) * 2^12

f32 = mybir.dt.float32
bf16 = mybir.dt.bfloat16
f16 = mybir.dt.float16

_COMPILED = None


def _build():
    nc = bacc.Bacc("TRN2", target_bir_lowering=False, debug=False,
                   num_devices=NCORES)

    xh = nc.dram_tensor("xh", [D, BC], f16, kind="ExternalInput").ap()
    xl = nc.dram_tensor("xl", [D, BC], f16, kind="ExternalInput").ap()
    eh = nc.dram_tensor("eh", [D, L], f16, kind="ExternalInput").ap()
    el = nc.dram_tensor("el", [D, L], f16, kind="ExternalInput").ap()
    lbh = nc.dram_tensor("lbh", [1, L], bf16, kind="ExternalInput").ap()
    lbl = nc.dram_tensor("lbl", [1, L], bf16, kind="ExternalInput").ap()
    dec_bf = nc.dram_tensor("dec_bf", [L, D], bf16, kind="ExternalInput").ap()
    pb_rep = nc.dram_tensor("pb_rep", [P, D], f32, kind="ExternalInput").ap()

    fl_o = nc.dram_tensor("fl", [BC, L], f32, kind="ExternalOutput").ap()
    lat_o = nc.dram_tensor("lat", [BC, L], f32, kind="ExternalOutput").ap()
    rec_o = nc.dram_tensor("rec", [BC, D], f32, kind="ExternalOutput").ap()
    recf_o = nc.dram_tensor("recf", [BC, D], f32, kind="ExternalOutput").ap()

    flbf_d = nc.dram_tensor("flbf_d", [BC, L], bf16).ap()
    latbf_d = nc.dram_tensor("latbf_d", [BC, L], bf16).ap()

    with tile.TileContext(nc) as tc:
        # ======== Phase A + A2 interleaved over m-tile pairs ========
        # Encoder is re-streamed once per pair (4x total); each pair's
        # DVE top-k work hides under the next pair's PE encode.
        with (
            tc.tile_pool(name="resA", bufs=1) as resA,
            tc.tile_pool(name="xp", bufs=2) as xp,
            tc.tile_pool(name="ehp", bufs=2) as ehp,
            tc.tile_pool(name="elp", bufs=1) as elp,
            tc.tile_pool(name="evA", bufs=2) as evA,
            tc.tile_pool(name="flp", bufs=1) as flp,
            tc.tile_pool(name="ck", bufs=2) as ck,
            tc.tile_pool(name="psA", bufs=2, space="PSUM") as psA,
        ):
            t_ones = resA.tile([1, P], bf16)
            nc.vector.memset(t_ones[:], 1.0)

            def topk_closures(mp):
                """Per-pair top-k work as a list of closures, to be
                interleaved into the next pair's encode loop (keeps the
                DVE queue alternating so PE never waits on evictions)."""
                cls = []
                for mi in range(2):
                    m = mp * 2 + mi
                    msl = slice(m * P, (m + 1) * P)
                    st = {}

                    def load(st=st, msl=msl):
                        fl_blk = flp.tile([P, L], f32, tag="flblk")
                        nc.sync.dma_start(fl_blk[:], fl_o[msl, :])
                        st["fl"] = fl_blk
                        st["mx"] = ck.tile([P, 32], f32, tag="maxes", name="maxes")
                    cls.append(load)
                    for r in range(4):
                        def rnd(st=st, r=r):
                            nc.vector.max(out=st["mx"][:, r * 8:(r + 1) * 8],
                                          in_=st["fl"][:])
                            nc.vector.match_replace(
                                out=st["fl"][:],
                                in_to_replace=st["mx"][:, r * 8:(r + 1) * 8],
                                in_values=st["fl"][:], imm_value=0.0)
                        cls.append(rnd)
                    for c in range(L // 2048):
                        def chunk(st=st, msl=msl, c=c):
                            csl = slice(c * 2048, (c + 1) * 2048)
                            flc = ck.tile([P, 2048], f32, tag="flc")
                            nc.sync.dma_start(flc[:], fl_o[msl, csl])
                            latc = ck.tile([P, 2048], f32, tag="latc")
                            nc.vector.tensor_sub(latc[:], flc[:],
                                                 st["fl"][:, csl])
                            nc.sync.dma_start(lat_o[msl, csl], latc[:])
                            lbfc = ck.tile([P, 2048], bf16, tag="lbfc")
                            nc.scalar.activation(
                                lbfc[:], latc[:],
                                mybir.ActivationFunctionType.Copy)
                            nc.sync.dma_start(latbf_d[msl, csl], lbfc[:])
                        cls.append(chunk)
                return cls

            pending = []
            for mp in range(MT // 2):
                t_xh = xp.tile([P, KC, 2, P], f16, tag="xh")
                nc.sync.dma_start(
                    t_xh[:], xh[:, mp * 256:(mp + 1) * 256].rearrange(
                        "(kc p) (mt mm) -> p kc mt mm", p=P, mm=P))
                t_xl = xp.tile([P, KC, 2, P], f16, tag="xl")
                nc.sync.dma_start(
                    t_xl[:], xl[:, mp * 256:(mp + 1) * 256].rearrange(
                        "(kc p) (mt mm) -> p kc mt mm", p=P, mm=P))
                done = 0
                for nt in range(NT):
                    nsl = slice(nt * 512, (nt + 1) * 512)
                    eh_blk = ehp.tile([P, KC, 512], f16, tag="ehblk")
                    nc.sync.dma_start(
                        eh_blk[:], eh[:, nsl].rearrange("(kc p) n -> p kc n", p=P))
                    el_blk = elp.tile([P, KC, 512], f16, tag="elblk")
                    nc.sync.dma_start(
                        el_blk[:], el[:, nsl].rearrange("(kc p) n -> p kc n", p=P))
                    t_lbh = ehp.tile([1, 512], bf16, tag="lbh")
                    nc.sync.dma_start(t_lbh[:], lbh[:, nsl])
                    t_lbl = ehp.tile([1, 512], bf16, tag="lbl")
                    nc.sync.dma_start(t_lbl[:], lbl[:, nsl])
                    for mi in range(2):
                        ps1 = psA.tile([P, 512], f32, tag="ps1")
                        ps2 = psA.tile([P, 512], f32, tag="ps2")
                        ps3 = psA.tile([P, 512], f32, tag="ps3")
                        for k in range(KC):
                            nc.tensor.matmul(ps1[:], t_xh[:, k, mi, :],
                                             eh_blk[:, k, :],
                                             start=(k == 0), stop=False)
                            nc.tensor.matmul(ps2[:], t_xl[:, k, mi, :],
                                             eh_blk[:, k, :],
                                             start=(k == 0), stop=(k == KC - 1))
                            nc.tensor.matmul(ps3[:], t_xh[:, k, mi, :],
                                             el_blk[:, k, :],
                                             start=(k == 0), stop=(k == KC - 1))
                        nc.tensor.matmul(ps1[:], t_ones[:, :], t_lbh[:, :],
                                         start=False, stop=False)
                        nc.tensor.matmul(ps1[:], t_ones[:, :], t_lbl[:, :],
                                         start=False, stop=True)
                        m = mp * 2 + mi
                        ta = evA.tile([P, 512], f32, tag="ta")
                        nc.scalar.activation(ta[:], ps2[:],
                                             mybir.ActivationFunctionType.Copy,
                                             scale=1.0 / XL_SCALE)
                        t2 = evA.tile([P, 512], f32, tag="t2")
                        nc.vector.scalar_tensor_tensor(
                            t2[:], ps3[:], 1.0 / EL_SCALE, ta[:],
                            op0=mybir.AluOpType.mult, op1=mybir.AluOpType.add)
                        t3 = evA.tile([P, 512], f32, tag="t3")
                        nc.vector.tensor_add(t3[:], ps1[:], t2[:])
                        o = evA.tile([P, 512], f32, tag="evict")
                        nc.scalar.activation(o[:], t3[:],
                                             mybir.ActivationFunctionType.Relu)
                        nc.sync.dma_start(fl_o[m * P:(m + 1) * P, nsl], o[:])
                        ob = evA.tile([P, 512], bf16, tag="evictbf")
                        nc.scalar.activation(ob[:], o[:],
                                             mybir.ActivationFunctionType.Copy)
                        nc.sync.dma_start(flbf_d[m * P:(m + 1) * P, nsl], ob[:])
                    # interleave previous pair's top-k work
                    want = (nt + 1) * len(pending) // NT if pending else 0
                    while done < want:
                        pending[done]()
                        done += 1
                while done < len(pending):
                    pending[done]()
                    done += 1
                pending = topk_closures(mp)
            for fn in pending:
                fn()

        # ======== Phase B: decode ========
        with (
            tc.tile_pool(name="resB", bufs=1) as resB,
            tc.tile_pool(name="acc", bufs=1) as accp,
            tc.tile_pool(name="decp", bufs=2) as decp,
            tc.tile_pool(name="tp", bufs=2) as tp,
            tc.tile_pool(name="outp", bufs=2) as outp,
            tc.tile_pool(name="psB", bufs=1, space="PSUM") as psB,
        ):
            t_pb = resB.tile([P, D], f32)
            nc.sync.dma_start(t_pb[:], pb_rep)

            for bh in range(2):
                rows = slice(bh * 512, (bh + 1) * 512)
                acc_r = accp.tile([P, 4, D], f32, tag="accr")
                acc_f = accp.tile([P, 4, D], f32, tag="accf")
                for kb in range(NKB):
                    d_blk = decp.tile([P, LKC, D], bf16, tag="decblk")
                    nc.sync.dma_start(
                        d_blk[:],
                        dec_bf[kb * LKB:(kb + 1) * LKB, :].rearrange(
                            "(kc p) d -> p kc d", p=P))
                    fT = tp.tile([P, LKC, 512], bf16, tag="fT")
                    lT = tp.tile([P, LKC, 512], bf16, tag="lT")
                    for j in range(LKC):
                        cl = slice(kb * LKB + j * P, kb * LKB + (j + 1) * P)
                        nc.sync.dma_start(fT[:, j, :], flbf_d[rows, cl],
                                          transpose=True)
                        nc.sync.dma_start(lT[:, j, :], latbf_d[rows, cl],
                                          transpose=True)
                    for mt in range(4):
                        mm = slice(mt * P, (mt + 1) * P)
                        for (tag, src, acc) in (("psf", fT, acc_f),
                                                ("psr", lT, acc_r)):
                            for h in range(2):
                                hsl = slice(h * 1024, (h + 1) * 1024)
                                ps = psB.tile([P, 1024], f32, tag=f"{tag}{h}")
                                for k in range(LKC):
                                    for n in range(2):
                                        nsl = slice(n * 512, (n + 1) * 512)
                                        gsl = slice(h * 1024 + n * 512,
                                                    h * 1024 + (n + 1) * 512)
                                        nc.tensor.matmul(
                                            ps[:, nsl], src[:, k, mm],
                                            d_blk[:, k, gsl],
                                            start=(k == 0), stop=(k == LKC - 1))
                                if kb == 0:
                                    nc.vector.tensor_copy(acc[:, mt, hsl], ps[:])
                                else:
                                    nc.vector.tensor_add(acc[:, mt, hsl],
                                                         acc[:, mt, hsl], ps[:])
                for mt in range(4):
                    grow = slice(bh * 512 + mt * P, bh * 512 + (mt + 1) * P)
                    o_f = outp.tile([P, D], f32, tag="of")
                    nc.vector.tensor_add(o_f[:], acc_f[:, mt, :], t_pb[:])
                    nc.sync.dma_start(recf_o[grow, :], o_f[:])
                    o_r = outp.tile([P, D], f32, tag="orr")
                    nc.vector.tensor_add(o_r[:], acc_r[:, mt, :], t_pb[:])
                    nc.sync.dma_start(rec_o[grow, :], o_r[:])

    nc.compile()
    return nc


def _split_f16(a, scale):
    hi = a.astype(np.float16)
    lo = ((a - hi.astype(np.float32)) * scale).astype(np.float16)
    return hi, lo


def kernel(x, pre_bias, encoder, latent_bias, decoder, k):
    global _COMPILED
    assert int(k) == 32
    x = np.asarray(x, np.float32)
    pre_bias = np.asarray(pre_bias, np.float32)
    encoder = np.ascontiguousarray(np.asarray(encoder, np.float32))
    latent_bias = np.asarray(latent_bias, np.float32)
    decoder = np.asarray(decoder, np.float32)

    xc = x - pre_bias                      # [B, D] fp32, same op as reference
    xcT = np.ascontiguousarray(xc.T)       # [D, B]
    xh_a, xl_a = _split_f16(xcT, XL_SCALE)
    eh_a, el_a = _split_f16(encoder, EL_SCALE)
    lbh_a = latent_bias.astype(ml_dtypes.bfloat16).reshape(1, L)
    lbl_a = (latent_bias - lbh_a.astype(np.float32)).astype(
        ml_dtypes.bfloat16).reshape(1, L)
    dec_bf = decoder.astype(ml_dtypes.bfloat16)
    pb_rep = np.ascontiguousarray(
        np.broadcast_to(pre_bias.reshape(1, D), (P, D)))

    if _COMPILED is None:
        _COMPILED = _build()
    nc = _COMPILED

    in_maps = []
    for c in range(NCORES):
        csl = slice(c * BC, (c + 1) * BC)
        in_maps.append(dict(
            xh=np.ascontiguousarray(xh_a[:, csl]),
            xl=np.ascontiguousarray(xl_a[:, csl]),
            eh=eh_a, el=el_a, lbh=lbh_a, lbl=lbl_a,
            dec_bf=dec_bf, pb_rep=pb_rep))
    kernel.last_in_maps = in_maps

    res = run_bass_kernel_spmd(nc, in_maps, list(range(NCORES)))
    outs = res.results

    rec = np.concatenate([outs[c]["rec"] for c in range(NCORES)], axis=0)
    lat = np.concatenate([outs[c]["lat"] for c in range(NCORES)], axis=0)
    recf = np.concatenate([outs[c]["recf"] for c in range(NCORES)], axis=0)
    fl = np.concatenate([outs[c]["fl"] for c in range(NCORES)], axis=0)
    return rec, lat, recf, fl


if __name__ == "__main__":
    rng = np.random.default_rng(0)
    x = rng.standard_normal((B, D), dtype=np.float32)
    pb = (rng.standard_normal(D) * 0.01).astype(np.float32)
    dec = rng.standard_normal((L, D), dtype=np.float32)
    dec = (dec * (0.1 / np.linalg.norm(dec, axis=-1, keepdims=True))).astype(np.float32)
    en = (dec.T + rng.standard_normal((D, L)) * 0.001).astype(np.float32)
    lb_ = (rng.standard_normal(L) * 0.01).astype(np.float32)
    out = kernel(x=x, pre_bias=pb, encoder=en, latent_bias=lb_, decoder=dec, k=32)
    for o in out:
        print(o.shape, o.dtype)


# revision 11
# speedup vs baseline: 1.1224x; 1.0653x over previous
"""TopK sparse autoencoder on 8 Trainium2 NeuronCores.

Data-parallel over batch: each core handles 1024 of 8192 rows.
Per core:
  Phase A : pre = xc @ enc + lb via fp16 triple-split matmuls
            (xh@eh + 2^-11*(xl@eh) + 2^-12*(xh@el), fp32-grade accuracy),
            relu -> fl written fp32 (output) + bf16 (decode scratch).
  Phase A2: per 128-row block: top-32 via 4x(max8 + match_replace),
            lat = fl - zapped; lat written fp32 (output) + bf16 (scratch).
  Phase B : rec_full = fl @ dec + pb, rec = lat @ dec + pb as bf16 matmuls
            contracting over L with xbar-transposed bf16 tiles.
"""
import os
import sys

for _p in ("/opt/trn_rl_repo", "/root/.axon_site/_ro/trn_rl_repo"):
    if os.path.isdir(_p) and _p not in sys.path:
        sys.path.append(_p)

import numpy as np
import ml_dtypes

import concourse.tile as tile
from concourse import bacc, mybir
from concourse.bass_utils import run_bass_kernel_spmd

P = 128
B = 8192          # full batch
D = 2048          # model dim
L = 16384         # latents
NCORES = 8
BC = B // NCORES  # rows per core = 1024
MT = BC // P      # m-tiles per core = 8
KC = D // P       # k-chunks for encode = 16
NT = L // 512     # n-tiles for encode = 32
LKB = 1024        # L-block for decode
NKB = L // LKB    # 16
LKC = LKB // P    # k-chunks per decode block = 8

XL_SCALE = 2.0 ** 11   # xl shipped as (xc - xh) * 2^11
EL_SCALE = 2.0 ** 12   # el shipped as (enc - eh) * 2^12

f32 = mybir.dt.float32
bf16 = mybir.dt.bfloat16
f16 = mybir.dt.float16

_COMPILED = None


def _build():
    nc = bacc.Bacc("TRN2", target_bir_lowering=False, debug=False,
                   num_devices=NCORES)

    xh = nc.dram_tensor("xh", [D, BC], f16, kind="ExternalInput").ap()
    xl = nc.dram_tensor("xl", [D, BC], f16, kind="ExternalInput").ap()
    eh = nc.dram_tensor("eh", [D, L], f16, kind="ExternalInput").ap()
    el = nc.dram_tensor("el", [D, L], f16, kind="ExternalInput").ap()
    lbh = nc.dram_tensor("lbh", [1, L], bf16, kind="ExternalInput").ap()
    lbl = nc.dram_tensor("lbl", [1, L], bf16, kind="ExternalInput").ap()
    dec_bf = nc.dram_tensor("dec_bf", [L, D], bf16, kind="ExternalInput").ap()
    pb_rep = nc.dram_tensor("pb_rep", [P, D], f32, kind="ExternalInput").ap()

    fl_o = nc.dram_tensor("fl", [BC, L], f32, kind="ExternalOutput").ap()
    lat_o = nc.dram_tensor("lat", [BC, L], f32, kind="ExternalOutput").ap()
    rec_o = nc.dram_tensor("rec", [BC, D], f32, kind="ExternalOutput").ap()
    recf_o = nc.dram_tensor("recf", [BC, D], f32, kind="ExternalOutput").ap()

    flbf_d = nc.dram_tensor("flbf_d", [BC, L], bf16).ap()
    latbf_d = nc.dram_tensor("latbf_d", [BC, L], bf16).ap()

    with tile.TileContext(nc) as tc:
        # ======== Phase A + A2 interleaved over m-tile pairs ========
        # Encoder is re-streamed once per pair (4x total); each pair's
        # DVE top-k work hides under the next pair's PE encode.
        with (
            tc.tile_pool(name="resA", bufs=1) as resA,
            tc.tile_pool(name="xp", bufs=2) as xp,
            tc.tile_pool(name="ehp", bufs=2) as ehp,
            tc.tile_pool(name="elp", bufs=2) as elp,
            tc.tile_pool(name="evA", bufs=2) as evA,
            tc.tile_pool(name="flp", bufs=1) as flp,
            tc.tile_pool(name="ck", bufs=2) as ck,
            tc.tile_pool(name="psA", bufs=2, space="PSUM") as psA,
        ):
            t_ones = resA.tile([1, P], bf16)
            nc.vector.memset(t_ones[:], 1.0)

            def topk_closures(mp):
                """Per-pair top-k work as a list of closures, to be
                interleaved into the next pair's encode loop (keeps the
                DVE queue alternating so PE never waits on evictions)."""
                cls = []
                for mi in range(2):
                    m = mp * 2 + mi
                    msl = slice(m * P, (m + 1) * P)
                    st = {}

                    def load(st=st, msl=msl):
                        fl_blk = flp.tile([P, L], f32, tag="flblk")
                        nc.sync.dma_start(fl_blk[:], fl_o[msl, :])
                        st["fl"] = fl_blk
                        st["mx"] = ck.tile([P, 32], f32, tag="maxes", name="maxes")
                    cls.append(load)
                    for r in range(4):
                        def rnd(st=st, r=r):
                            nc.vector.max(out=st["mx"][:, r * 8:(r + 1) * 8],
                                          in_=st["fl"][:])
                            nc.vector.match_replace(
                                out=st["fl"][:],
                                in_to_replace=st["mx"][:, r * 8:(r + 1) * 8],
                                in_values=st["fl"][:], imm_value=0.0)
                        cls.append(rnd)
                    for c in range(L // 1024):
                        def chunk(st=st, msl=msl, c=c):
                            csl = slice(c * 1024, (c + 1) * 1024)
                            flc = ck.tile([P, 1024], f32, tag="flc")
                            nc.gpsimd.dma_start(flc[:], fl_o[msl, csl])
                            latc = ck.tile([P, 1024], f32, tag="latc")
                            nc.vector.tensor_sub(latc[:], flc[:],
                                                 st["fl"][:, csl])
                            nc.gpsimd.dma_start(lat_o[msl, csl], latc[:])
                            lbfc = ck.tile([P, 1024], bf16, tag="lbfc")
                            nc.scalar.activation(
                                lbfc[:], latc[:],
                                mybir.ActivationFunctionType.Copy)
                            nc.gpsimd.dma_start(latbf_d[msl, csl], lbfc[:])
                        cls.append(chunk)
                return cls

            pending = []
            for mp in range(MT // 2):
                t_xh = xp.tile([P, KC, 2, P], f16, tag="xh")
                nc.sync.dma_start(
                    t_xh[:], xh[:, mp * 256:(mp + 1) * 256].rearrange(
                        "(kc p) (mt mm) -> p kc mt mm", p=P, mm=P))
                t_xl = xp.tile([P, KC, 2, P], f16, tag="xl")
                nc.sync.dma_start(
                    t_xl[:], xl[:, mp * 256:(mp + 1) * 256].rearrange(
                        "(kc p) (mt mm) -> p kc mt mm", p=P, mm=P))
                done = 0
                for nt in range(NT):
                    nsl = slice(nt * 512, (nt + 1) * 512)
                    eh_blk = ehp.tile([P, KC, 512], f16, tag="ehblk")
                    nc.sync.dma_start(
                        eh_blk[:], eh[:, nsl].rearrange("(kc p) n -> p kc n", p=P))
                    el_blk = elp.tile([P, KC, 512], f16, tag="elblk")
                    nc.sync.dma_start(
                        el_blk[:], el[:, nsl].rearrange("(kc p) n -> p kc n", p=P))
                    t_lbh = ehp.tile([1, 512], bf16, tag="lbh")
                    nc.sync.dma_start(t_lbh[:], lbh[:, nsl])
                    t_lbl = ehp.tile([1, 512], bf16, tag="lbl")
                    nc.sync.dma_start(t_lbl[:], lbl[:, nsl])
                    for mi in range(2):
                        ps1 = psA.tile([P, 512], f32, tag="ps1", bufs=3)
                        ps2 = psA.tile([P, 512], f32, tag="ps2")
                        ps3 = psA.tile([P, 512], f32, tag="ps3")
                        for k in range(KC):
                            nc.tensor.matmul(ps1[:], t_xh[:, k, mi, :],
                                             eh_blk[:, k, :],
                                             start=(k == 0), stop=False)
                            nc.tensor.matmul(ps2[:], t_xl[:, k, mi, :],
                                             eh_blk[:, k, :],
                                             start=(k == 0), stop=(k == KC - 1))
                            nc.tensor.matmul(ps3[:], t_xh[:, k, mi, :],
                                             el_blk[:, k, :],
                                             start=(k == 0), stop=(k == KC - 1))
                        nc.tensor.matmul(ps1[:], t_ones[:, :], t_lbh[:, :],
                                         start=False, stop=False)
                        nc.tensor.matmul(ps1[:], t_ones[:, :], t_lbl[:, :],
                                         start=False, stop=True)
                        m = mp * 2 + mi
                        ta = evA.tile([P, 512], f32, tag="ta")
                        nc.scalar.activation(ta[:], ps2[:],
                                             mybir.ActivationFunctionType.Copy,
                                             scale=1.0 / XL_SCALE)
                        t2 = evA.tile([P, 512], f32, tag="t2")
                        nc.vector.scalar_tensor_tensor(
                            t2[:], ps3[:], 1.0 / EL_SCALE, ta[:],
                            op0=mybir.AluOpType.mult, op1=mybir.AluOpType.add)
                        t3 = evA.tile([P, 512], f32, tag="t3")
                        nc.vector.tensor_add(t3[:], ps1[:], t2[:])
                        o = evA.tile([P, 512], f32, tag="evict")
                        nc.scalar.activation(o[:], t3[:],
                                             mybir.ActivationFunctionType.Relu)
                        nc.gpsimd.dma_start(fl_o[m * P:(m + 1) * P, nsl], o[:])
                        ob = evA.tile([P, 512], bf16, tag="evictbf")
                        nc.scalar.activation(ob[:], o[:],
                                             mybir.ActivationFunctionType.Copy)
                        nc.gpsimd.dma_start(flbf_d[m * P:(m + 1) * P, nsl], ob[:])
                    # interleave previous pair's top-k work
                    want = (nt + 1) * len(pending) // NT if pending else 0
                    while done < want:
                        pending[done]()
                        done += 1
                while done < len(pending):
                    pending[done]()
                    done += 1
                pending = topk_closures(mp)
            for fn in pending:
                fn()

        # ======== Phase B: decode ========
        with (
            tc.tile_pool(name="resB", bufs=1) as resB,
            tc.tile_pool(name="acc", bufs=1) as accp,
            tc.tile_pool(name="decp", bufs=2) as decp,
            tc.tile_pool(name="tp", bufs=2) as tp,
            tc.tile_pool(name="outp", bufs=2) as outp,
            tc.tile_pool(name="psB", bufs=1, space="PSUM") as psB,
        ):
            t_pb = resB.tile([P, D], f32)
            nc.sync.dma_start(t_pb[:], pb_rep)

            for bh in range(2):
                rows = slice(bh * 512, (bh + 1) * 512)
                acc_r = accp.tile([P, 4, D], f32, tag="accr")
                acc_f = accp.tile([P, 4, D], f32, tag="accf")
                for kb in range(NKB):
                    d_blk = decp.tile([P, LKC, D], bf16, tag="decblk")
                    nc.sync.dma_start(
                        d_blk[:],
                        dec_bf[kb * LKB:(kb + 1) * LKB, :].rearrange(
                            "(kc p) d -> p kc d", p=P))
                    fT = tp.tile([P, LKC, 512], bf16, tag="fT")
                    lT = tp.tile([P, LKC, 512], bf16, tag="lT")
                    for j in range(LKC):
                        cl = slice(kb * LKB + j * P, kb * LKB + (j + 1) * P)
                        nc.sync.dma_start(fT[:, j, :], flbf_d[rows, cl],
                                          transpose=True)
                        nc.sync.dma_start(lT[:, j, :], latbf_d[rows, cl],
                                          transpose=True)
                    for mt in range(4):
                        mm = slice(mt * P, (mt + 1) * P)
                        for (tag, src, acc) in (("psf", fT, acc_f),
                                                ("psr", lT, acc_r)):
                            for h in range(2):
                                hsl = slice(h * 1024, (h + 1) * 1024)
                                ps = psB.tile([P, 1024], f32, tag=f"{tag}{h}")
                                for k in range(LKC):
                                    for n in range(2):
                                        nsl = slice(n * 512, (n + 1) * 512)
                                        gsl = slice(h * 1024 + n * 512,
                                                    h * 1024 + (n + 1) * 512)
                                        nc.tensor.matmul(
                                            ps[:, nsl], src[:, k, mm],
                                            d_blk[:, k, gsl],
                                            start=(k == 0), stop=(k == LKC - 1))
                                if kb == 0:
                                    nc.vector.tensor_copy(acc[:, mt, hsl], ps[:])
                                else:
                                    nc.vector.tensor_add(acc[:, mt, hsl],
                                                         acc[:, mt, hsl], ps[:])
                for mt in range(4):
                    grow = slice(bh * 512 + mt * P, bh * 512 + (mt + 1) * P)
                    o_f = outp.tile([P, D], f32, tag="of")
                    nc.vector.tensor_add(o_f[:], acc_f[:, mt, :], t_pb[:])
                    nc.sync.dma_start(recf_o[grow, :], o_f[:])
                    o_r = outp.tile([P, D], f32, tag="orr")
                    nc.vector.tensor_add(o_r[:], acc_r[:, mt, :], t_pb[:])
                    nc.sync.dma_start(rec_o[grow, :], o_r[:])

    nc.compile()
    return nc


def _split_f16(a, scale):
    hi = a.astype(np.float16)
    lo = ((a - hi.astype(np.float32)) * scale).astype(np.float16)
    return hi, lo


def kernel(x, pre_bias, encoder, latent_bias, decoder, k):
    global _COMPILED
    assert int(k) == 32
    x = np.asarray(x, np.float32)
    pre_bias = np.asarray(pre_bias, np.float32)
    encoder = np.ascontiguousarray(np.asarray(encoder, np.float32))
    latent_bias = np.asarray(latent_bias, np.float32)
    decoder = np.asarray(decoder, np.float32)

    xc = x - pre_bias                      # [B, D] fp32, same op as reference
    xcT = np.ascontiguousarray(xc.T)       # [D, B]
    xh_a, xl_a = _split_f16(xcT, XL_SCALE)
    eh_a, el_a = _split_f16(encoder, EL_SCALE)
    lbh_a = latent_bias.astype(ml_dtypes.bfloat16).reshape(1, L)
    lbl_a = (latent_bias - lbh_a.astype(np.float32)).astype(
        ml_dtypes.bfloat16).reshape(1, L)
    dec_bf = decoder.astype(ml_dtypes.bfloat16)
    pb_rep = np.ascontiguousarray(
        np.broadcast_to(pre_bias.reshape(1, D), (P, D)))

    if _COMPILED is None:
        _COMPILED = _build()
    nc = _COMPILED

    in_maps = []
    for c in range(NCORES):
        csl = slice(c * BC, (c + 1) * BC)
        in_maps.append(dict(
            xh=np.ascontiguousarray(xh_a[:, csl]),
            xl=np.ascontiguousarray(xl_a[:, csl]),
            eh=eh_a, el=el_a, lbh=lbh_a, lbl=lbl_a,
            dec_bf=dec_bf, pb_rep=pb_rep))
    kernel.last_in_maps = in_maps

    res = run_bass_kernel_spmd(nc, in_maps, list(range(NCORES)))
    outs = res.results

    rec = np.concatenate([outs[c]["rec"] for c in range(NCORES)], axis=0)
    lat = np.concatenate([outs[c]["lat"] for c in range(NCORES)], axis=0)
    recf = np.concatenate([outs[c]["recf"] for c in range(NCORES)], axis=0)
    fl = np.concatenate([outs[c]["fl"] for c in range(NCORES)], axis=0)
    return rec, lat, recf, fl


if __name__ == "__main__":
    rng = np.random.default_rng(0)
    x = rng.standard_normal((B, D), dtype=np.float32)
    pb = (rng.standard_normal(D) * 0.01).astype(np.float32)
    dec = rng.standard_normal((L, D), dtype=np.float32)
    dec = (dec * (0.1 / np.linalg.norm(dec, axis=-1, keepdims=True))).astype(np.float32)
    en = (dec.T + rng.standard_normal((D, L)) * 0.001).astype(np.float32)
    lb_ = (rng.standard_normal(L) * 0.01).astype(np.float32)
    out = kernel(x=x, pre_bias=pb, encoder=en, latent_bias=lb_, decoder=dec, k=32)
    for o in out:
        print(o.shape, o.dtype)
